# revision 39
# baseline (speedup 1.0000x reference)
"""ConvCaps dynamic-routing kernel for 8 TRN2 NeuronCores.

Strategy (data-parallel over batch B=8, one batch element per core):
  - Everything in bf16 (tolerance is 2e-2; bf16 lands ~1e-3).
  - Grouped 3x3 conv (groups=D=32) as one bf16 matmul per group per
    pixel tile: stationary = im2col patches [72, npx], moving = weights
    [72, 512], PSUM fp32 -> u tile in SBUF as bf16 [px, D, c, d].
  - iter-0 s (zero prior => uniform c) comes free from TensorE: a second
    moving pass per group accumulates sum_D u into one PSUM bank.
  - Routing einsum contractions run on the Vector engine as bf16
    tensor_tensor multiplies (2x mode) + in-place binary-tree adds
    (2x mode) instead of TENSOR_REDUCE (1x, ~1.6 cyc/elem measured).
  - Output s [px, (c,d)] is PE-transposed to [(c,d), px] and DMA'd out.
"""

import numpy as np
from contextlib import ExitStack

import ml_dtypes

import concourse.bacc as bacc
import concourse.bass as bass
import concourse.tile as tile
import concourse.mybir as mybir
from concourse.bass_utils import run_bass_kernel_spmd
from concourse.masks import make_identity

F32 = mybir.dt.float32
BF16 = mybir.dt.bfloat16
AF = mybir.ActivationFunctionType

B = 8
C_IN, D_IN = 8, 32
C_OUT, D_OUT = 16, 32
KS = 3
H = W = 32
HO = WO = 30
NPX = HO * WO                 # 900 output pixels per batch element
KDIM = C_IN * KS * KS         # 72 = contraction dim of the conv matmul
CD = C_OUT * D_OUT            # 512 out-channels per group
ITERS = 3
P = 128
EPS = 1e-8
# 7 main tiles of 124 px (row-crossing; 5-row im2col windows) leave a
# 32-px remainder that fits a 4-way d-split on 128 lanes (FD 4096).
PX_TILES = [(124 * t, 124) for t in range(7)]
MICRO_PX0, MICRO_NPX = 868, 32


def _tree_reduce_flat(nc, tmp, pxs, n, dst):
    """In-place binary tree sum of tmp[pxs, :n] down to dst[pxs] (n//? -> 512).

    tmp is a [P, n] view; halves n each level with bf16 tensor_tensor adds
    (2x DVE mode) until 1024, then the final add writes into dst (512 wide).
    """
    while n > 1024:
        h = n // 2
        nc.vector.tensor_add(tmp[pxs, 0:h], tmp[pxs, 0:h], tmp[pxs, h:n])
        n = h
    nc.vector.tensor_add(dst, tmp[pxs, 0:512], tmp[pxs, 512:1024])


def _tree_reduce_c(nc, tmp, pxs, dst):
    """Sum tmp[pxs, D, c, d] over c (16) -> dst [pxs, D, d] (in-place tree)."""
    c = C_OUT
    while c > 2:
        h = c // 2
        nc.vector.tensor_add(tmp[pxs, :, 0:h, :], tmp[pxs, :, 0:h, :],
                             tmp[pxs, :, h:c, :])
        c = h
    nc.vector.tensor_add(dst, tmp[pxs, :, 0, :], tmp[pxs, :, 1, :])


def _body(ctx, tc, xb, wt, b0, swp, out, zero_prior):
    nc = tc.nc
    consts = ctx.enter_context(tc.tile_pool(name="consts", bufs=1))
    wpool = ctx.enter_context(tc.tile_pool(name="wpool", bufs=1))
    x9pool = ctx.enter_context(tc.tile_pool(name="x9pool", bufs=2))
    upool = ctx.enter_context(tc.tile_pool(name="upool", bufs=2))
    s0pool = ctx.enter_context(tc.tile_pool(name="s0pool", bufs=2))
    rpool = ctx.enter_context(tc.tile_pool(name="rpool", bufs=2))
    tmppool = ctx.enter_context(tc.tile_pool(name="tmppool", bufs=2))
    opool = ctx.enter_context(tc.tile_pool(name="opool", bufs=2))
    psum_c = ctx.enter_context(tc.tile_pool(name="psum_c", bufs=4, space="PSUM"))
    psum_s = ctx.enter_context(tc.tile_pool(name="psum_s", bufs=1, space="PSUM"))
    psum_t = ctx.enter_context(tc.tile_pool(name="psum_t", bufs=2, space="PSUM"))

    # trigger the weight load first so the transfers start before the
    # gpsimd identity/memset setup delays the queues; chunked + spread over
    # all three DMA-capable engines so the first conv matmul waits ~300KB,
    # not 2.4MB.
    w_sb = wpool.tile([KDIM, D_IN * CD], BF16)
    WCH = D_IN * CD // 8

    def _w_chunks(lo, hi, engs):
        for ci in range(lo, hi):
            eng = engs[ci % len(engs)]
            eng.dma_start(w_sb[:, ci * WCH:(ci + 1) * WCH],
                          wt[:, ci * WCH:(ci + 1) * WCH])
    # all early chunks on gpsimd: sync/scalar must stay clear for tile-0's
    # im2col so the first conv isn't serialized behind weight bytes
    _w_chunks(0, 3, (nc.gpsimd,))
    ident = consts.tile([P, P], BF16)
    make_identity(nc, ident)
    # warm the PE clock (HAM releases the throttle after ~3.4us of
    # sustained activity) while the first im2col DMAs are in flight
    for i in range(10):
        pw = psum_s.tile([P, P], BF16, tag="warm", bufs=1)
        nc.tensor.transpose(pw[:], ident[:], ident[:])
    for cval in (EPS, 1.0, 32.0):
        cb = consts.tile([P, 1], F32, tag=f"const_{cval}")
        nc.gpsimd.memset(cb[:], cval)
        nc.const_aps.aps[(F32, cval)] = cb[:]
    swp_sb = consts.tile([P, P], BF16, tag="swp")
    nc.sync.dma_start(swp_sb[:], swp)
    if not zero_prior:
        b0_sb = consts.tile([P, D_IN, D_OUT], BF16)
        nc.sync.dma_start(b0_sb[:], b0)

    tiles = list(PX_TILES)
    if not zero_prior:
        tiles.append((MICRO_PX0, MICRO_NPX))  # normal-layout fallback
    for ti, (px0, npx) in enumerate(tiles):
        r0, o = px0 // 30, px0 % 30
        nr = min(5, H - KS + 1 - r0)
        pxs = slice(0, npx)
        # all tiles take the free PE s0 pass: at startup the conv rate is
        # weight-DMA-bound, so the extra moving pass costs no critical path
        mm_s0 = zero_prior

        # ---- im2col: 9 shifted window loads; partition k = (kh*3+kw)*8 + C
        # x is [C, H, W, D] on the host so each (kh,kw) window is a dense
        # (w,d) run -> one DMA per k-position with ~2KB packets.
        x9 = x9pool.tile([KDIM, 5, WO, D_IN], BF16, tag="x9")
        di = 0
        for kh in range(KS):
            for kw in range(KS):
                kk = kh * KS + kw
                eng = nc.scalar if (ti == 0 and di % 2) else nc.sync
                eng.dma_start(
                    x9[kk * C_IN:(kk + 1) * C_IN, 0:nr, :, :],
                    xb[:, r0 + kh:r0 + kh + nr, kw:kw + WO, :],
                )
                di += 1
        if ti == 0:
            # rest of the weights on the otherwise-idle gpsimd queue so
            # tile-1's im2col doesn't wait behind them on sync/scalar
            _w_chunks(3, 8, (nc.gpsimd,))

        # ---- grouped conv: one bf16 matmul per group; a second moving pass
        # accumulates sum_D u into ps0 (free iter-0 s when prior is zero).
        u_t = upool.tile([P, D_IN, C_OUT, D_OUT], BF16, tag="u")
        s0_sb = s0pool.tile([P, C_OUT, D_OUT], BF16, tag="s0")
        if mm_s0:
            ps0 = psum_s.tile([P, CD], F32, tag="ps0")
        for g in range(D_IN):
            pu = psum_c.tile([P, CD], F32, tag="pu")
            xg = x9[:, 0:nr, :, g].rearrange("k r c -> k (r c)")[:, o:o + npx]
            nc.tensor.matmul(
                pu[pxs, :],
                xg,
                w_sb[:, g * CD:(g + 1) * CD],
                start=True, stop=True,
            )
            if mm_s0:
                nc.tensor.matmul(
                    ps0[pxs, :],
                    xg,
                    w_sb[:, g * CD:(g + 1) * CD],
                    start=(g == 0), stop=(g == D_IN - 1),
                    skip_group_check=True,
                )
            if ti == 0 and g % 2:
                # startup: vector is idle, let it share the PSUM drain
                nc.vector.tensor_copy(u_t[pxs, g], pu[pxs, :])
            else:
                nc.scalar.copy(u_t[pxs, g], pu[pxs, :])
        if mm_s0:
            nc.scalar.activation(s0_sb[pxs], ps0[pxs, :], AF.Copy,
                                 scale=1.0 / D_IN)

        # ---- routing state tiles
        b_t = rpool.tile([P, D_IN, D_OUT], BF16, tag="b")
        c_e = rpool.tile([P, D_IN, D_OUT], BF16, tag="ce")
        c_t = rpool.tile([P, D_IN, D_OUT], BF16, tag="c")
        ak_t = rpool.tile([P, D_IN, D_OUT], BF16, tag="ak")
        s_t = rpool.tile([P, C_OUT, D_OUT], BF16, tag="s")
        sq_t = rpool.tile([P, C_OUT, D_OUT], F32, tag="sq")
        v_t = rpool.tile([P, C_OUT, D_OUT], BF16, tag="v")
        n2_t = rpool.tile([P, D_OUT], F32, tag="n2")
        r_t = rpool.tile([P, D_OUT], F32, tag="r")
        q_t = rpool.tile([P, D_OUT], F32, tag="q")
        f_t = rpool.tile([P, D_OUT], F32, tag="f")
        rsum = rpool.tile([P, D_IN], F32, tag="rsum")
        tmp = tmppool.tile([P, D_IN, C_OUT, D_OUT], BF16, tag="tmp")
        tmp_flat = tmp[:].rearrange("p a b c -> p (a b c)")

        if not zero_prior:
            nc.scalar.copy(b_t[pxs], b0_sb[pxs])

        for it in range(ITERS):
            first = it == 0
            last = it == ITERS - 1
            uniform0 = first and zero_prior

            # softmax over d (no max-subtraction: logits are O(1) here)
            if not uniform0:
                src = b0_sb if (first and not zero_prior) else b_t
                # exp in two D-halves so the first reduce overlaps the
                # second half's ACT latency (exposed at light-pipeline
                # stretches near startup)
                hD = D_IN // 2
                nc.scalar.activation(c_e[pxs, 0:hD], src[pxs, 0:hD], AF.Exp)
                nc.vector.reduce_sum(rsum[pxs, 0:hD], c_e[pxs, 0:hD],
                                     axis=mybir.AxisListType.X)
                nc.scalar.activation(c_e[pxs, hD:D_IN], src[pxs, hD:D_IN],
                                     AF.Exp)
                nc.vector.reduce_sum(rsum[pxs, hD:D_IN], c_e[pxs, hD:D_IN],
                                     axis=mybir.AxisListType.X)
                nc.vector.reciprocal(rsum[pxs], rsum[pxs])
                nc.vector.tensor_mul(
                    c_t[pxs], c_e[pxs],
                    rsum[pxs].unsqueeze(2).broadcast_to((npx, D_IN, D_OUT)))

            # s[c,d] = sum_D c[D,d] * u[D,c,d]
            s_scale = 1.0
            if uniform0:
                s_cur = s0_sb
                if not mm_s0:
                    # tile 0: tree-sum u over D on the vector engine; defer
                    # the 1/32 mean scale into the squash (free in ACT args)
                    u_flat = u_t[:].rearrange("p a b c -> p (a b c)")
                    h = D_IN * CD // 2
                    nc.vector.tensor_add(tmp_flat[pxs, 0:h],
                                         u_flat[pxs, 0:h],
                                         u_flat[pxs, h:2 * h])
                    _tree_reduce_flat(nc, tmp_flat, pxs, h, s0_sb[pxs])
                    s_scale = 1.0 / D_IN
            else:
                s_cur = s_t
                nc.vector.tensor_mul(
                    tmp[pxs], u_t[pxs],
                    c_t[pxs].unsqueeze(2)
                    .broadcast_to((npx, D_IN, C_OUT, D_OUT)))
                _tree_reduce_flat(nc, tmp_flat, pxs, D_IN * CD, s_t[pxs])

            if last:
                break

            # squash over c: v = s * n2 / ((1+n2) * sqrt(n2+eps)); when
            # s_cur holds 32*s, fold the 1/32 into the Square scale and the
            # 32 into q so v = s_cur * f comes out right.
            nc.scalar.activation(sq_t[pxs], s_cur[pxs], AF.Square,
                                 scale=s_scale)
            nc.vector.reduce_sum(n2_t[pxs], sq_t[pxs].transpose([0, 2, 1]),
                                 axis=mybir.AxisListType.X)
            nc.scalar.activation(r_t[pxs], n2_t[pxs], AF.Sqrt, bias=EPS)
            if s_scale == 1.0:
                nc.scalar.add(q_t[pxs], n2_t[pxs], 1.0)
            else:
                nc.scalar.activation(q_t[pxs], n2_t[pxs], AF.Identity,
                                     bias=float(D_IN), scale=float(D_IN))
            nc.vector.tensor_mul(f_t[pxs], q_t[pxs], r_t[pxs])
            nc.vector.reciprocal(f_t[pxs], f_t[pxs])
            nc.vector.tensor_mul(f_t[pxs], f_t[pxs], n2_t[pxs])
            nc.vector.tensor_mul(
                v_t[pxs], s_cur[pxs],
                f_t[pxs].unsqueeze(1).broadcast_to((npx, C_OUT, D_OUT)))

            # b[D,d] += sum_c u[D,c,d] * v[c,d]
            nc.vector.tensor_mul(
                tmp[pxs], u_t[pxs],
                v_t[pxs].unsqueeze(1)
                .broadcast_to((npx, D_IN, C_OUT, D_OUT)))
            if uniform0:
                # b was zero: write the reduction straight into b
                _tree_reduce_c(nc, tmp, pxs, b_t[pxs])
            else:
                _tree_reduce_c(nc, tmp, pxs, ak_t[pxs])
                nc.vector.tensor_add(b_t[pxs], b_t[pxs], ak_t[pxs])

        # ---- write s out as [(c,d), px]: PE transpose in 128-row blocks
        s_flat = s_t[:].rearrange("p a b -> p (a b)")
        for blk in range(CD // P):
            pt = psum_t.tile([P, 124], BF16, tag="pt")
            nc.tensor.transpose(
                pt[:, pxs], s_flat[pxs, blk * P:(blk + 1) * P],
                ident[pxs, pxs])
            ob = opool.tile([P, P], F32, tag="ob")
            nc.scalar.copy(ob[:, pxs], pt[:, pxs])
            nc.sync.dma_start(
                out[blk * P:(blk + 1) * P, px0:px0 + npx],
                ob[:, pxs])

    if zero_prior:
        # ---- micro tile (px 868..899, 32 px): 4-way d-split, quarter q on
        # lanes 32q..32q+31 carrying d = 8q..8q+7, so every op runs at
        # FD 4096 instead of a full tile's 16384.  All 128 lanes are used,
        # no junk hygiene needed.  Only the softmax denominator crosses
        # lanes: one PE matmul against comb[k,m] = (k == m mod 32).
        px0, npx = MICRO_PX0, MICRO_NPX
        # 3-row im2col window starting one row early: the quarter-3 conv
        # matmul needs 32 extra (stale) columns on its left (see below)
        r0 = px0 // 30 - 1
        o = px0 - r0 * 30
        nr = 3
        NL, DL = 128, 8
        lx = slice(0, NL)

        x9 = x9pool.tile([KDIM, 5, WO, D_IN], BF16, tag="x9")
        for kh in range(KS):
            for kw in range(KS):
                kk = kh * KS + kw
                nc.sync.dma_start(
                    x9[kk * C_IN:(kk + 1) * C_IN, 0:nr, :, :],
                    xb[:, r0 + kh:r0 + kh + nr, kw:kw + WO, :],
                )

        u_t = upool.tile([P, D_IN, C_OUT, D_OUT], BF16, tag="u")
        s0_sb = s0pool.tile([P, C_OUT, D_OUT], BF16, tag="s0")
        u7 = u_t[:].rearrange("p a b c -> p (a b c)")[:, 0:D_IN * C_OUT * DL] \
            .rearrange("p (a b c) -> p a b c", a=D_IN, b=C_OUT, c=DL)
        s0_7 = s0_sb[:].rearrange("p a b -> p (a b)")[:, 0:C_OUT * DL] \
            .rearrange("p (a b) -> p a b", a=C_OUT)
        tmp = tmppool.tile([P, D_IN, C_OUT, D_OUT], BF16, tag="tmp")
        tmp7 = tmp[:].rearrange("p a b c -> p (a b c)")[:, 0:D_IN * C_OUT * DL] \
            .rearrange("p (a b c) -> p a b c", a=D_IN, b=C_OUT, c=DL)
        for g in range(D_IN):
            wg = w_sb[:, g * CD:(g + 1) * CD].rearrange(
                "k (c d) -> k c d", c=C_OUT)
            xm = x9[:, 0:nr, :, g].rearrange("k r c -> k (r c)")
            xg = xm[:, o:o + npx]
            pu = psum_c.tile([P, CD], F32, tag="pu")
            # partition bases are limited to {0, 32, 64}: quarter 3 can't
            # start at 96, so issue it as a base-64 M=64 matmul whose lhsT
            # carries 32 stale columns on the left (polluting [64:96]),
            # then let quarter 2's start=True matmul overwrite that range.
            nc.tensor.matmul(
                pu[64:128, 0:C_OUT * DL],
                xm[:, o - 32:o + npx], wg[:, :, 3 * DL:4 * DL],
                start=True, stop=True, skip_group_check=True,
            )
            for h in range(3):
                ls = slice(32 * h, 32 * h + npx)
                nc.tensor.matmul(
                    pu[ls, 0:C_OUT * DL],
                    xg, wg[:, :, h * DL:(h + 1) * DL],
                    start=True, stop=True, skip_group_check=True,
                )
            nc.scalar.copy(u7[lx, g], pu[lx, 0:C_OUT * DL])
        # s0 = sum_D u via a small DVE tree (the PE s0 pass is incompatible
        # with the overwrite trick above); 1/32 folds into the squash
        nc.vector.tensor_add(tmp7[lx, 0:16], u7[lx, 0:16], u7[lx, 16:32])
        dd = 16
        while dd > 2:
            hh = dd // 2
            nc.vector.tensor_add(tmp7[lx, 0:hh], tmp7[lx, 0:hh],
                                 tmp7[lx, hh:dd])
            dd = hh
        nc.vector.tensor_add(s0_7[lx], tmp7[lx, 0], tmp7[lx, 1])

        b_t = rpool.tile([P, D_IN, D_OUT], BF16, tag="b")
        c_e = rpool.tile([P, D_IN, D_OUT], BF16, tag="ce")
        c_t = rpool.tile([P, D_IN, D_OUT], BF16, tag="c")
        ak_t = rpool.tile([P, D_IN, D_OUT], BF16, tag="ak")
        s_t = rpool.tile([P, C_OUT, D_OUT], BF16, tag="s")
        sq_t = rpool.tile([P, C_OUT, D_OUT], F32, tag="sq")
        v_t = rpool.tile([P, C_OUT, D_OUT], BF16, tag="v")
        n2_t = rpool.tile([P, D_OUT], F32, tag="n2")
        r_t = rpool.tile([P, D_OUT], F32, tag="r")
        q_t = rpool.tile([P, D_OUT], F32, tag="q")
        f_t = rpool.tile([P, D_OUT], F32, tag="f")
        rsum = rpool.tile([P, D_IN], F32, tag="rsum")
        rs7 = rpool.tile([P, D_IN], BF16, tag="rs7")

        def dsplit(t, a):
            return t[:].rearrange("p a b -> p (a b)")[:, 0:a * DL] \
                .rearrange("p (a b) -> p a b", a=a)
        b7, ce7, ct7, ak7 = (dsplit(t, D_IN) for t in (b_t, c_e, c_t, ak_t))
        s7, sq7, v7 = (dsplit(t, C_OUT) for t in (s_t, sq_t, v_t))

        for it in range(ITERS):
            first = it == 0
            last = it == ITERS - 1

            if not first:
                nc.scalar.activation(ce7[lx], b7[lx], AF.Exp)
                with nc.allow_low_precision(
                        reason="bf16 softmax partial sums feed a bf16 "
                               "PE matmul; affects 32/900 px only"):
                    nc.vector.reduce_sum(rs7[lx], ce7[lx],
                                         axis=mybir.AxisListType.X)
                psx = psum_c.tile([P, CD], F32, tag="pu")
                nc.tensor.matmul(psx[lx, 0:D_IN], swp_sb[lx, lx], rs7[lx],
                                 start=True, stop=True)
                nc.vector.reciprocal(rsum[lx], psx[lx, 0:D_IN])
                nc.vector.tensor_mul(
                    ct7[lx], ce7[lx],
                    rsum[lx].unsqueeze(2).broadcast_to((NL, D_IN, DL)))

            if first:
                s_cur = s0_7
            else:
                s_cur = s7
                nc.vector.tensor_mul(
                    tmp7[lx], u7[lx],
                    ct7[lx].unsqueeze(2)
                    .broadcast_to((NL, D_IN, C_OUT, DL)))
                dd = D_IN
                while dd > 2:
                    hh = dd // 2
                    nc.vector.tensor_add(tmp7[lx, 0:hh], tmp7[lx, 0:hh],
                                         tmp7[lx, hh:dd])
                    dd = hh
                nc.vector.tensor_add(s7[lx], tmp7[lx, 0], tmp7[lx, 1])

            if last:
                break

            s_scale = 1.0 / D_IN if first else 1.0
            nc.scalar.activation(sq7[lx], s_cur[lx], AF.Square,
                                 scale=s_scale)
            nc.vector.reduce_sum(n2_t[lx, 0:DL], sq7[lx].transpose([0, 2, 1]),
                                 axis=mybir.AxisListType.X)
            nc.scalar.activation(r_t[lx, 0:DL], n2_t[lx, 0:DL], AF.Sqrt,
                                 bias=EPS)
            if first:
                nc.scalar.activation(q_t[lx, 0:DL], n2_t[lx, 0:DL],
                                     AF.Identity, bias=float(D_IN),
                                     scale=float(D_IN))
            else:
                nc.scalar.add(q_t[lx, 0:DL], n2_t[lx, 0:DL], 1.0)
            nc.vector.tensor_mul(f_t[lx, 0:DL], q_t[lx, 0:DL], r_t[lx, 0:DL])
            nc.vector.reciprocal(f_t[lx, 0:DL], f_t[lx, 0:DL])
            nc.vector.tensor_mul(f_t[lx, 0:DL], f_t[lx, 0:DL],
                                 n2_t[lx, 0:DL])
            nc.vector.tensor_mul(
                v7[lx], s_cur[lx],
                f_t[lx, 0:DL].unsqueeze(1).broadcast_to((NL, C_OUT, DL)))

            nc.vector.tensor_mul(
                tmp7[lx], u7[lx],
                v7[lx].unsqueeze(1).broadcast_to((NL, D_IN, C_OUT, DL)))
            cc = C_OUT
            while cc > 2:
                hh = cc // 2
                nc.vector.tensor_add(tmp7[lx, :, 0:hh], tmp7[lx, :, 0:hh],
                                     tmp7[lx, :, hh:cc])
                cc = hh
            if first:
                nc.vector.tensor_add(b7[lx], tmp7[lx, :, 0], tmp7[lx, :, 1])
            else:
                nc.vector.tensor_add(ak7[lx], tmp7[lx, :, 0], tmp7[lx, :, 1])
                nc.vector.tensor_add(b7[lx], b7[lx], ak7[lx])

        # ---- out: one transpose gives rows (c, dl); lane quarter q holds
        # d = 8q + dl -> four DMAs, one per quarter.
        s7f = s_t[:].rearrange("p a b -> p (a b)")
        outv = out.rearrange("(c d) n -> c d n", c=C_OUT)
        pt = psum_s.tile([P, P], BF16, tag="warm", bufs=1)
        nc.tensor.transpose(pt[:, lx], s7f[lx, 0:P], ident[lx, lx])
        ob = opool.tile([P, P], F32, tag="ob")
        nc.scalar.copy(ob[:, lx], pt[:, lx])
        for q in range(4):
            eng = (nc.sync, nc.scalar, nc.gpsimd, nc.scalar)[q]
            eng.dma_start(
                outv[:, q * DL:(q + 1) * DL, px0:px0 + npx],
                ob[:, 32 * q:32 * q + npx])


_CACHE = {}


def _build(zero_prior: bool):
    key = ("v4", zero_prior)
    if key in _CACHE:
        return _CACHE[key]
    nc = bacc.Bacc("TRN2", target_bir_lowering=False, debug=False,
                   enable_asserts=True, num_devices=B)
    xb = nc.dram_tensor("xb", [C_IN, H, W, D_IN], BF16,
                        kind="ExternalInput").ap()
    wt = nc.dram_tensor("wt", [KDIM, D_IN * CD], BF16,
                        kind="ExternalInput").ap()
    b0 = nc.dram_tensor("b0", [P, D_IN, D_OUT], BF16,
                        kind="ExternalInput").ap()
    swp = nc.dram_tensor("swp", [P, P], BF16, kind="ExternalInput").ap()
    out = nc.dram_tensor("out", [CD, NPX], F32, kind="ExternalOutput").ap()
    with tile.TileContext(nc) as tc:
        with ExitStack() as ctx:
            _body(ctx, tc, xb, wt, b0, swp, out, zero_prior)
    nc.compile()
    _CACHE[key] = nc
    return nc


def _prep_inputs(x, conv_w, prior):
    # weights: rows (D,c,d) x (C,kh,kw) -> [k=(kh,kw,C), (D,c,d)]
    wt = conv_w.reshape(D_IN, C_OUT, D_OUT, C_IN, KS, KS)
    wt = np.ascontiguousarray(wt.transpose(4, 5, 3, 0, 1, 2)).reshape(KDIM, D_IN * CD)
    wt = wt.astype(ml_dtypes.bfloat16)
    pb = np.broadcast_to(prior.reshape(D_IN, D_OUT), (P, D_IN, D_OUT))
    b0 = np.ascontiguousarray(pb).astype(ml_dtypes.bfloat16)
    # comb[k, m] = 1 for k == m (mod 32): one matmul against this sums
    # each lane's softmax denominator across its three partner quarters
    ii = np.arange(P)
    comb = (ii[:, None] % 32 == ii[None, :] % 32)
    comb = comb.astype(np.float32).astype(ml_dtypes.bfloat16)
    # [B, C, D, H, W] -> [B, C, H, W, D] so im2col windows are dense runs
    xbf = np.ascontiguousarray(x.transpose(0, 1, 3, 4, 2)).astype(
        ml_dtypes.bfloat16)
    in_maps = [
        {"xb": xbf[b], "wt": wt, "b0": b0, "swp": comb}
        for b in range(B)
    ]
    return in_maps


def kernel(x, conv_w, prior):
    x = np.asarray(x, dtype=np.float32)
    conv_w = np.asarray(conv_w, dtype=np.float32)
    prior = np.asarray(prior, dtype=np.float32)
    zero_prior = not np.any(prior)
    nc = _build(zero_prior)
    in_maps = _prep_inputs(x, conv_w, prior)
    res = run_bass_kernel_spmd(nc, in_maps, list(range(B)))
    outs = [res.results[b]["out"].reshape(C_OUT, D_OUT, HO, WO)
            for b in range(B)]
    return np.stack(outs, axis=0).astype(np.float32)



# revision 40
# speedup vs baseline: 1.0040x; 1.0040x over previous
"""ConvCaps dynamic-routing kernel for 8 TRN2 NeuronCores.

Strategy (data-parallel over batch B=8, one batch element per core):
  - Everything in bf16 (tolerance is 2e-2; bf16 lands ~1e-3).
  - Grouped 3x3 conv (groups=D=32) as one bf16 matmul per group per
    pixel tile: stationary = im2col patches [72, npx], moving = weights
    [72, 512], PSUM fp32 -> u tile in SBUF as bf16 [px, D, c, d].
  - iter-0 s (zero prior => uniform c) comes free from TensorE: a second
    moving pass per group accumulates sum_D u into one PSUM bank.
  - Routing einsum contractions run on the Vector engine as bf16
    tensor_tensor multiplies (2x mode) + in-place binary-tree adds
    (2x mode) instead of TENSOR_REDUCE (1x, ~1.6 cyc/elem measured).
  - Output s [px, (c,d)] is PE-transposed to [(c,d), px] and DMA'd out.
"""

import numpy as np
from contextlib import ExitStack

import ml_dtypes

import concourse.bacc as bacc
import concourse.bass as bass
import concourse.tile as tile
import concourse.mybir as mybir
from concourse.bass_utils import run_bass_kernel_spmd
from concourse.masks import make_identity

F32 = mybir.dt.float32
BF16 = mybir.dt.bfloat16
AF = mybir.ActivationFunctionType

B = 8
C_IN, D_IN = 8, 32
C_OUT, D_OUT = 16, 32
KS = 3
H = W = 32
HO = WO = 30
NPX = HO * WO                 # 900 output pixels per batch element
KDIM = C_IN * KS * KS         # 72 = contraction dim of the conv matmul
CD = C_OUT * D_OUT            # 512 out-channels per group
ITERS = 3
P = 128
EPS = 1e-8
# 7 main tiles of 124 px (row-crossing; 5-row im2col windows) leave a
# 32-px remainder that fits a 4-way d-split on 128 lanes (FD 4096).
PX_TILES = [(124 * t, 124) for t in range(7)]
MICRO_PX0, MICRO_NPX = 868, 32


def _tree_reduce_flat(nc, tmp, pxs, n, dst):
    """In-place binary tree sum of tmp[pxs, :n] down to dst[pxs] (n//? -> 512).

    tmp is a [P, n] view; halves n each level with bf16 tensor_tensor adds
    (2x DVE mode) until 1024, then the final add writes into dst (512 wide).
    """
    while n > 1024:
        h = n // 2
        nc.vector.tensor_add(tmp[pxs, 0:h], tmp[pxs, 0:h], tmp[pxs, h:n])
        n = h
    nc.vector.tensor_add(dst, tmp[pxs, 0:512], tmp[pxs, 512:1024])


def _tree_reduce_c(nc, tmp, pxs, dst):
    """Sum tmp[pxs, D, c, d] over c (16) -> dst [pxs, D, d] (in-place tree)."""
    c = C_OUT
    while c > 2:
        h = c // 2
        nc.vector.tensor_add(tmp[pxs, :, 0:h, :], tmp[pxs, :, 0:h, :],
                             tmp[pxs, :, h:c, :])
        c = h
    nc.vector.tensor_add(dst, tmp[pxs, :, 0, :], tmp[pxs, :, 1, :])


def _body(ctx, tc, xb, wt, b0, swp, out, zero_prior):
    nc = tc.nc
    consts = ctx.enter_context(tc.tile_pool(name="consts", bufs=1))
    wpool = ctx.enter_context(tc.tile_pool(name="wpool", bufs=1))
    x9pool = ctx.enter_context(tc.tile_pool(name="x9pool", bufs=2))
    upool = ctx.enter_context(tc.tile_pool(name="upool", bufs=2))
    s0pool = ctx.enter_context(tc.tile_pool(name="s0pool", bufs=2))
    rpool = ctx.enter_context(tc.tile_pool(name="rpool", bufs=2))
    tmppool = ctx.enter_context(tc.tile_pool(name="tmppool", bufs=2))
    opool = ctx.enter_context(tc.tile_pool(name="opool", bufs=2))
    psum_c = ctx.enter_context(tc.tile_pool(name="psum_c", bufs=4, space="PSUM"))
    psum_s = ctx.enter_context(tc.tile_pool(name="psum_s", bufs=1, space="PSUM"))
    psum_t = ctx.enter_context(tc.tile_pool(name="psum_t", bufs=2, space="PSUM"))

    # trigger the weight load first so the transfers start before the
    # gpsimd identity/memset setup delays the queues; chunked + spread over
    # all three DMA-capable engines so the first conv matmul waits ~300KB,
    # not 2.4MB.
    w_sb = wpool.tile([KDIM, D_IN * CD], BF16)
    WCH = D_IN * CD // 8

    def _w_chunks(lo, hi, engs):
        for ci in range(lo, hi):
            eng = engs[ci % len(engs)]
            eng.dma_start(w_sb[:, ci * WCH:(ci + 1) * WCH],
                          wt[:, ci * WCH:(ci + 1) * WCH])
    # all early chunks on gpsimd: sync/scalar must stay clear for tile-0's
    # im2col so the first conv isn't serialized behind weight bytes
    _w_chunks(0, 3, (nc.gpsimd,))
    ident = consts.tile([P, P], BF16)
    make_identity(nc, ident)
    # warm the PE clock (HAM releases the throttle after ~3.4us of
    # sustained activity) while the first im2col DMAs are in flight
    for i in range(10):
        pw = psum_s.tile([P, P], BF16, tag="warm", bufs=1)
        nc.tensor.transpose(pw[:], ident[:], ident[:])
    for cval in (EPS, 1.0, 32.0):
        cb = consts.tile([P, 1], F32, tag=f"const_{cval}")
        nc.gpsimd.memset(cb[:], cval)
        nc.const_aps.aps[(F32, cval)] = cb[:]
    swp_sb = consts.tile([P, P], BF16, tag="swp")
    nc.sync.dma_start(swp_sb[:], swp)
    if not zero_prior:
        b0_sb = consts.tile([P, D_IN, D_OUT], BF16)
        nc.sync.dma_start(b0_sb[:], b0)

    tiles = list(PX_TILES)
    if not zero_prior:
        tiles.append((MICRO_PX0, MICRO_NPX))  # normal-layout fallback
    for ti, (px0, npx) in enumerate(tiles):
        r0, o = px0 // 30, px0 % 30
        nr = min(5, H - KS + 1 - r0)
        pxs = slice(0, npx)
        # all tiles take the free PE s0 pass: at startup the conv rate is
        # weight-DMA-bound, so the extra moving pass costs no critical path
        mm_s0 = zero_prior

        # ---- im2col: 9 shifted window loads; partition k = (kh*3+kw)*8 + C
        # x is [C, H, W, D] on the host so each (kh,kw) window is a dense
        # (w,d) run -> one DMA per k-position with ~2KB packets.
        x9 = x9pool.tile([KDIM, 5, WO, D_IN], BF16, tag="x9")
        di = 0
        for kh in range(KS):
            for kw in range(KS):
                kk = kh * KS + kw
                eng = nc.scalar if (ti == 0 and di % 2) else nc.sync
                eng.dma_start(
                    x9[kk * C_IN:(kk + 1) * C_IN, 0:nr, :, :],
                    xb[:, r0 + kh:r0 + kh + nr, kw:kw + WO, :],
                )
                di += 1
        if ti == 0:
            # rest of the weights on the otherwise-idle gpsimd queue so
            # tile-1's im2col doesn't wait behind them on sync/scalar
            _w_chunks(3, 8, (nc.gpsimd,))

        # ---- grouped conv: one bf16 matmul per group; a second moving pass
        # accumulates sum_D u into ps0 (free iter-0 s when prior is zero).
        u_t = upool.tile([P, D_IN, C_OUT, D_OUT], BF16, tag="u")
        s0_sb = s0pool.tile([P, C_OUT, D_OUT], BF16, tag="s0")
        if mm_s0:
            ps0 = psum_s.tile([P, CD], F32, tag="ps0")
        for g in range(D_IN):
            pu = psum_c.tile([P, CD], F32, tag="pu")
            xg = x9[:, 0:nr, :, g].rearrange("k r c -> k (r c)")[:, o:o + npx]
            nc.tensor.matmul(
                pu[pxs, :],
                xg,
                w_sb[:, g * CD:(g + 1) * CD],
                start=True, stop=True,
            )
            if mm_s0:
                nc.tensor.matmul(
                    ps0[pxs, :],
                    xg,
                    w_sb[:, g * CD:(g + 1) * CD],
                    start=(g == 0), stop=(g == D_IN - 1),
                    skip_group_check=True,
                )
            if ti == 0 and g % 2:
                # startup: vector is idle, let it share the PSUM drain
                nc.vector.tensor_copy(u_t[pxs, g], pu[pxs, :])
            else:
                nc.scalar.copy(u_t[pxs, g], pu[pxs, :])
        if mm_s0:
            nc.scalar.activation(s0_sb[pxs], ps0[pxs, :], AF.Copy,
                                 scale=1.0 / D_IN)

        # ---- routing state tiles
        b_t = rpool.tile([P, D_IN, D_OUT], BF16, tag="b")
        c_e = rpool.tile([P, D_IN, D_OUT], BF16, tag="ce")
        c_t = rpool.tile([P, D_IN, D_OUT], BF16, tag="c")
        ak_t = rpool.tile([P, D_IN, D_OUT], BF16, tag="ak")
        s_t = rpool.tile([P, C_OUT, D_OUT], BF16, tag="s")
        sq_t = rpool.tile([P, C_OUT, D_OUT], F32, tag="sq")
        v_t = rpool.tile([P, C_OUT, D_OUT], BF16, tag="v")
        n2_t = rpool.tile([P, D_OUT], F32, tag="n2")
        r_t = rpool.tile([P, D_OUT], F32, tag="r")
        q_t = rpool.tile([P, D_OUT], F32, tag="q")
        f_t = rpool.tile([P, D_OUT], F32, tag="f")
        rsum = rpool.tile([P, D_IN], F32, tag="rsum")
        tmp = tmppool.tile([P, D_IN, C_OUT, D_OUT], BF16, tag="tmp")
        tmp_flat = tmp[:].rearrange("p a b c -> p (a b c)")

        if not zero_prior:
            nc.scalar.copy(b_t[pxs], b0_sb[pxs])

        for it in range(ITERS):
            first = it == 0
            last = it == ITERS - 1
            uniform0 = first and zero_prior

            # softmax over d (no max-subtraction: logits are O(1) here)
            if not uniform0:
                src = b0_sb if (first and not zero_prior) else b_t
                nc.scalar.activation(c_e[pxs], src[pxs], AF.Exp)
                nc.vector.reduce_sum(rsum[pxs], c_e[pxs],
                                     axis=mybir.AxisListType.X)
                nc.vector.reciprocal(rsum[pxs], rsum[pxs])
                nc.vector.tensor_mul(
                    c_t[pxs], c_e[pxs],
                    rsum[pxs].unsqueeze(2).broadcast_to((npx, D_IN, D_OUT)))

            # s[c,d] = sum_D c[D,d] * u[D,c,d]
            s_scale = 1.0
            if uniform0:
                s_cur = s0_sb
                if not mm_s0:
                    # tile 0: tree-sum u over D on the vector engine; defer
                    # the 1/32 mean scale into the squash (free in ACT args)
                    u_flat = u_t[:].rearrange("p a b c -> p (a b c)")
                    h = D_IN * CD // 2
                    nc.vector.tensor_add(tmp_flat[pxs, 0:h],
                                         u_flat[pxs, 0:h],
                                         u_flat[pxs, h:2 * h])
                    _tree_reduce_flat(nc, tmp_flat, pxs, h, s0_sb[pxs])
                    s_scale = 1.0 / D_IN
            else:
                s_cur = s_t
                nc.vector.tensor_mul(
                    tmp[pxs], u_t[pxs],
                    c_t[pxs].unsqueeze(2)
                    .broadcast_to((npx, D_IN, C_OUT, D_OUT)))
                _tree_reduce_flat(nc, tmp_flat, pxs, D_IN * CD, s_t[pxs])

            if last:
                break

            # squash over c: v = s * n2 / ((1+n2) * sqrt(n2+eps)); when
            # s_cur holds 32*s, fold the 1/32 into the Square scale and the
            # 32 into q so v = s_cur * f comes out right.
            nc.scalar.activation(sq_t[pxs], s_cur[pxs], AF.Square,
                                 scale=s_scale)
            nc.vector.reduce_sum(n2_t[pxs], sq_t[pxs].transpose([0, 2, 1]),
                                 axis=mybir.AxisListType.X)
            nc.scalar.activation(r_t[pxs], n2_t[pxs], AF.Sqrt, bias=EPS)
            if s_scale == 1.0:
                nc.scalar.add(q_t[pxs], n2_t[pxs], 1.0)
            else:
                nc.scalar.activation(q_t[pxs], n2_t[pxs], AF.Identity,
                                     bias=float(D_IN), scale=float(D_IN))
            nc.vector.tensor_mul(f_t[pxs], q_t[pxs], r_t[pxs])
            nc.vector.reciprocal(f_t[pxs], f_t[pxs])
            nc.vector.tensor_mul(f_t[pxs], f_t[pxs], n2_t[pxs])
            nc.vector.tensor_mul(
                v_t[pxs], s_cur[pxs],
                f_t[pxs].unsqueeze(1).broadcast_to((npx, C_OUT, D_OUT)))

            # b[D,d] += sum_c u[D,c,d] * v[c,d]
            nc.vector.tensor_mul(
                tmp[pxs], u_t[pxs],
                v_t[pxs].unsqueeze(1)
                .broadcast_to((npx, D_IN, C_OUT, D_OUT)))
            if uniform0:
                # b was zero: write the reduction straight into b
                _tree_reduce_c(nc, tmp, pxs, b_t[pxs])
            else:
                _tree_reduce_c(nc, tmp, pxs, ak_t[pxs])
                nc.vector.tensor_add(b_t[pxs], b_t[pxs], ak_t[pxs])

        # ---- write s out as [(c,d), px]: PE transpose in 128-row blocks
        s_flat = s_t[:].rearrange("p a b -> p (a b)")
        for blk in range(CD // P):
            pt = psum_t.tile([P, 124], BF16, tag="pt")
            nc.tensor.transpose(
                pt[:, pxs], s_flat[pxs, blk * P:(blk + 1) * P],
                ident[pxs, pxs])
            ob = opool.tile([P, P], F32, tag="ob")
            nc.scalar.copy(ob[:, pxs], pt[:, pxs])
            nc.sync.dma_start(
                out[blk * P:(blk + 1) * P, px0:px0 + npx],
                ob[:, pxs])

    if zero_prior:
        # ---- micro tile (px 868..899, 32 px): 4-way d-split, quarter q on
        # lanes 32q..32q+31 carrying d = 8q..8q+7, so every op runs at
        # FD 4096 instead of a full tile's 16384.  All 128 lanes are used,
        # no junk hygiene needed.  Only the softmax denominator crosses
        # lanes: one PE matmul against comb[k,m] = (k == m mod 32).
        px0, npx = MICRO_PX0, MICRO_NPX
        # 3-row im2col window starting one row early: the quarter-3 conv
        # matmul needs 32 extra (stale) columns on its left (see below)
        r0 = px0 // 30 - 1
        o = px0 - r0 * 30
        nr = 3
        NL, DL = 128, 8
        lx = slice(0, NL)

        x9 = x9pool.tile([KDIM, 5, WO, D_IN], BF16, tag="x9")
        for kh in range(KS):
            for kw in range(KS):
                kk = kh * KS + kw
                nc.sync.dma_start(
                    x9[kk * C_IN:(kk + 1) * C_IN, 0:nr, :, :],
                    xb[:, r0 + kh:r0 + kh + nr, kw:kw + WO, :],
                )

        u_t = upool.tile([P, D_IN, C_OUT, D_OUT], BF16, tag="u")
        s0_sb = s0pool.tile([P, C_OUT, D_OUT], BF16, tag="s0")
        u7 = u_t[:].rearrange("p a b c -> p (a b c)")[:, 0:D_IN * C_OUT * DL] \
            .rearrange("p (a b c) -> p a b c", a=D_IN, b=C_OUT, c=DL)
        s0_7 = s0_sb[:].rearrange("p a b -> p (a b)")[:, 0:C_OUT * DL] \
            .rearrange("p (a b) -> p a b", a=C_OUT)
        tmp = tmppool.tile([P, D_IN, C_OUT, D_OUT], BF16, tag="tmp")
        tmp7 = tmp[:].rearrange("p a b c -> p (a b c)")[:, 0:D_IN * C_OUT * DL] \
            .rearrange("p (a b c) -> p a b c", a=D_IN, b=C_OUT, c=DL)
        for g in range(D_IN):
            wg = w_sb[:, g * CD:(g + 1) * CD].rearrange(
                "k (c d) -> k c d", c=C_OUT)
            xm = x9[:, 0:nr, :, g].rearrange("k r c -> k (r c)")
            xg = xm[:, o:o + npx]
            pu = psum_c.tile([P, CD], F32, tag="pu")
            # partition bases are limited to {0, 32, 64}: quarter 3 can't
            # start at 96, so issue it as a base-64 M=64 matmul whose lhsT
            # carries 32 stale columns on the left (polluting [64:96]),
            # then let quarter 2's start=True matmul overwrite that range.
            nc.tensor.matmul(
                pu[64:128, 0:C_OUT * DL],
                xm[:, o - 32:o + npx], wg[:, :, 3 * DL:4 * DL],
                start=True, stop=True, skip_group_check=True,
            )
            for h in range(3):
                ls = slice(32 * h, 32 * h + npx)
                nc.tensor.matmul(
                    pu[ls, 0:C_OUT * DL],
                    xg, wg[:, :, h * DL:(h + 1) * DL],
                    start=True, stop=True, skip_group_check=True,
                )
            nc.scalar.copy(u7[lx, g], pu[lx, 0:C_OUT * DL])
        # s0 = sum_D u via a small DVE tree (the PE s0 pass is incompatible
        # with the overwrite trick above); 1/32 folds into the squash
        nc.vector.tensor_add(tmp7[lx, 0:16], u7[lx, 0:16], u7[lx, 16:32])
        dd = 16
        while dd > 2:
            hh = dd // 2
            nc.vector.tensor_add(tmp7[lx, 0:hh], tmp7[lx, 0:hh],
                                 tmp7[lx, hh:dd])
            dd = hh
        nc.vector.tensor_add(s0_7[lx], tmp7[lx, 0], tmp7[lx, 1])

        b_t = rpool.tile([P, D_IN, D_OUT], BF16, tag="b")
        c_e = rpool.tile([P, D_IN, D_OUT], BF16, tag="ce")
        c_t = rpool.tile([P, D_IN, D_OUT], BF16, tag="c")
        ak_t = rpool.tile([P, D_IN, D_OUT], BF16, tag="ak")
        s_t = rpool.tile([P, C_OUT, D_OUT], BF16, tag="s")
        sq_t = rpool.tile([P, C_OUT, D_OUT], F32, tag="sq")
        v_t = rpool.tile([P, C_OUT, D_OUT], BF16, tag="v")
        n2_t = rpool.tile([P, D_OUT], F32, tag="n2")
        r_t = rpool.tile([P, D_OUT], F32, tag="r")
        q_t = rpool.tile([P, D_OUT], F32, tag="q")
        f_t = rpool.tile([P, D_OUT], F32, tag="f")
        rsum = rpool.tile([P, D_IN], F32, tag="rsum")
        rs7 = rpool.tile([P, D_IN], BF16, tag="rs7")

        def dsplit(t, a):
            return t[:].rearrange("p a b -> p (a b)")[:, 0:a * DL] \
                .rearrange("p (a b) -> p a b", a=a)
        b7, ce7, ct7, ak7 = (dsplit(t, D_IN) for t in (b_t, c_e, c_t, ak_t))
        s7, sq7, v7 = (dsplit(t, C_OUT) for t in (s_t, sq_t, v_t))

        for it in range(ITERS):
            first = it == 0
            last = it == ITERS - 1

            if not first:
                nc.scalar.activation(ce7[lx], b7[lx], AF.Exp)
                with nc.allow_low_precision(
                        reason="bf16 softmax partial sums feed a bf16 "
                               "PE matmul; affects 32/900 px only"):
                    nc.vector.reduce_sum(rs7[lx], ce7[lx],
                                         axis=mybir.AxisListType.X)
                psx = psum_c.tile([P, CD], F32, tag="pu")
                nc.tensor.matmul(psx[lx, 0:D_IN], swp_sb[lx, lx], rs7[lx],
                                 start=True, stop=True)
                nc.vector.reciprocal(rsum[lx], psx[lx, 0:D_IN])
                nc.vector.tensor_mul(
                    ct7[lx], ce7[lx],
                    rsum[lx].unsqueeze(2).broadcast_to((NL, D_IN, DL)))

            if first:
                s_cur = s0_7
            else:
                s_cur = s7
                nc.vector.tensor_mul(
                    tmp7[lx], u7[lx],
                    ct7[lx].unsqueeze(2)
                    .broadcast_to((NL, D_IN, C_OUT, DL)))
                dd = D_IN
                while dd > 2:
                    hh = dd // 2
                    nc.vector.tensor_add(tmp7[lx, 0:hh], tmp7[lx, 0:hh],
                                         tmp7[lx, hh:dd])
                    dd = hh
                nc.vector.tensor_add(s7[lx], tmp7[lx, 0], tmp7[lx, 1])

            if last:
                break

            s_scale = 1.0 / D_IN if first else 1.0
            nc.scalar.activation(sq7[lx], s_cur[lx], AF.Square,
                                 scale=s_scale)
            nc.vector.reduce_sum(n2_t[lx, 0:DL], sq7[lx].transpose([0, 2, 1]),
                                 axis=mybir.AxisListType.X)
            nc.scalar.activation(r_t[lx, 0:DL], n2_t[lx, 0:DL], AF.Sqrt,
                                 bias=EPS)
            if first:
                nc.scalar.activation(q_t[lx, 0:DL], n2_t[lx, 0:DL],
                                     AF.Identity, bias=float(D_IN),
                                     scale=float(D_IN))
            else:
                nc.scalar.add(q_t[lx, 0:DL], n2_t[lx, 0:DL], 1.0)
            nc.vector.tensor_mul(f_t[lx, 0:DL], q_t[lx, 0:DL], r_t[lx, 0:DL])
            nc.vector.reciprocal(f_t[lx, 0:DL], f_t[lx, 0:DL])
            nc.vector.tensor_mul(f_t[lx, 0:DL], f_t[lx, 0:DL],
                                 n2_t[lx, 0:DL])
            nc.vector.tensor_mul(
                v7[lx], s_cur[lx],
                f_t[lx, 0:DL].unsqueeze(1).broadcast_to((NL, C_OUT, DL)))

            nc.vector.tensor_mul(
                tmp7[lx], u7[lx],
                v7[lx].unsqueeze(1).broadcast_to((NL, D_IN, C_OUT, DL)))
            cc = C_OUT
            while cc > 2:
                hh = cc // 2
                nc.vector.tensor_add(tmp7[lx, :, 0:hh], tmp7[lx, :, 0:hh],
                                     tmp7[lx, :, hh:cc])
                cc = hh
            if first:
                nc.vector.tensor_add(b7[lx], tmp7[lx, :, 0], tmp7[lx, :, 1])
            else:
                nc.vector.tensor_add(ak7[lx], tmp7[lx, :, 0], tmp7[lx, :, 1])
                nc.vector.tensor_add(b7[lx], b7[lx], ak7[lx])

        # ---- out: one transpose gives rows (c, dl); lane quarter q holds
        # d = 8q + dl -> four DMAs, one per quarter.
        s7f = s_t[:].rearrange("p a b -> p (a b)")
        outv = out.rearrange("(c d) n -> c d n", c=C_OUT)
        pt = psum_s.tile([P, P], BF16, tag="warm", bufs=1)
        nc.tensor.transpose(pt[:, lx], s7f[lx, 0:P], ident[lx, lx])
        ob = opool.tile([P, P], F32, tag="ob")
        nc.scalar.copy(ob[:, lx], pt[:, lx])
        for q in range(4):
            eng = (nc.sync, nc.scalar, nc.gpsimd, nc.scalar)[q]
            eng.dma_start(
                outv[:, q * DL:(q + 1) * DL, px0:px0 + npx],
                ob[:, 32 * q:32 * q + npx])


_CACHE = {}


def _build(zero_prior: bool):
    key = ("v4", zero_prior)
    if key in _CACHE:
        return _CACHE[key]
    nc = bacc.Bacc("TRN2", target_bir_lowering=False, debug=False,
                   enable_asserts=True, num_devices=B)
    xb = nc.dram_tensor("xb", [C_IN, H, W, D_IN], BF16,
                        kind="ExternalInput").ap()
    wt = nc.dram_tensor("wt", [KDIM, D_IN * CD], BF16,
                        kind="ExternalInput").ap()
    b0 = nc.dram_tensor("b0", [P, D_IN, D_OUT], BF16,
                        kind="ExternalInput").ap()
    swp = nc.dram_tensor("swp", [P, P], BF16, kind="ExternalInput").ap()
    out = nc.dram_tensor("out", [CD, NPX], F32, kind="ExternalOutput").ap()
    with tile.TileContext(nc) as tc:
        with ExitStack() as ctx:
            _body(ctx, tc, xb, wt, b0, swp, out, zero_prior)
    nc.compile()
    _CACHE[key] = nc
    return nc


def _prep_inputs(x, conv_w, prior):
    # weights: rows (D,c,d) x (C,kh,kw) -> [k=(kh,kw,C), (D,c,d)]
    wt = conv_w.reshape(D_IN, C_OUT, D_OUT, C_IN, KS, KS)
    wt = np.ascontiguousarray(wt.transpose(4, 5, 3, 0, 1, 2)).reshape(KDIM, D_IN * CD)
    wt = wt.astype(ml_dtypes.bfloat16)
    pb = np.broadcast_to(prior.reshape(D_IN, D_OUT), (P, D_IN, D_OUT))
    b0 = np.ascontiguousarray(pb).astype(ml_dtypes.bfloat16)
    # comb[k, m] = 1 for k == m (mod 32): one matmul against this sums
    # each lane's softmax denominator across its three partner quarters
    ii = np.arange(P)
    comb = (ii[:, None] % 32 == ii[None, :] % 32)
    comb = comb.astype(np.float32).astype(ml_dtypes.bfloat16)
    # [B, C, D, H, W] -> [B, C, H, W, D] so im2col windows are dense runs
    xbf = np.ascontiguousarray(x.transpose(0, 1, 3, 4, 2)).astype(
        ml_dtypes.bfloat16)
    in_maps = [
        {"xb": xbf[b], "wt": wt, "b0": b0, "swp": comb}
        for b in range(B)
    ]
    return in_maps


def kernel(x, conv_w, prior):
    x = np.asarray(x, dtype=np.float32)
    conv_w = np.asarray(conv_w, dtype=np.float32)
    prior = np.asarray(prior, dtype=np.float32)
    zero_prior = not np.any(prior)
    nc = _build(zero_prior)
    in_maps = _prep_inputs(x, conv_w, prior)
    res = run_bass_kernel_spmd(nc, in_maps, list(range(B)))
    outs = [res.results[b]["out"].reshape(C_OUT, D_OUT, HO, WO)
            for b in range(B)]
    return np.stack(outs, axis=0).astype(np.float32)

